# revision 29
# baseline (speedup 1.0000x reference)
"""Trainium2 Bass kernel for nn_MixtureOfExperts_89172111000183.

Strategy: expert-parallel across the 8 NeuronCores (E == n_cores == 8).
Core e holds only expert e's weights (pre-transposed on the host so the
contraction dim lands on SBUF partitions) and computes, for all T=2048
tokens:

    logits = x @ w_router.T + b          (full router, fp32 matmul)
    top-2 dispatch coefficient for THIS core's expert via a max/2nd-max
    trick (softmax + top-k renorm collapses to a sigmoid of m1-m2)
    partial_e = dispatch_e * (gelu(x @ w1[e].T) @ w2[e].T)

The host sums the 8 partials (the all-reduce step of expert-parallel
MoE) and takes router logits from core 0. The MLP matmuls run as
float32r (full-rate fp32 streaming on the PE); the router matmul runs
plain fp32 so top-2 selection matches the fp32 reference.

Per-core dispatch uses a column-swap trick: core e receives w_router.T
with columns 0<->e swapped, so "my expert's logit" is always column 0
of the on-device logits; core 0's swap is the identity, so its logits
output (the only one kept) is unpermuted.
"""

import numpy as np

# Problem shapes (hardcoded per contract).
B, S, D, H, E = 2, 1024, 512, 1024, 8
T = B * S            # 2048 tokens
P = 128              # SBUF partitions
TCH = 512            # token chunk (matmul free dim)
NT = T // TCH        # 4 token chunks
NTS = T // P         # 16 token subtiles
ND = D // P          # 4 contraction chunks for fc1/router
NH = H // P          # 8 h chunks
N_CORES = 8

_CACHE = {}


def _patch_tile_drain():
    """This walrus build's CTRL_NO struct encodes at most one sync-wait,
    but TileContext's kernel-tail drain carries one wait per live
    semaphore. Split the waits onto single-wait NOPs before the drain."""
    import concourse.tile as tile
    import concourse.mybir as mybir
    from concourse.vector_clock import ScopedClock

    if getattr(tile.TileContext, "_drain_patch_applied", False):
        return

    def _drain_and_barrier(self, tick_clock, wait_clock):
        # Slim tail: the stock version runs TWO all-engine EVSEM
        # butterflies around the sem clears (~9-17us). Replace with a
        # single join semaphore: SP waits for every proc's final tick
        # (carrier nops cover all DMA-completion sems) and drains its
        # queues, each compute engine signals stream-end, and gpsimd
        # clears the semaphore range once all five engines have signaled
        # so the next execution of the NEFF starts from zeroed sems.
        nc = self.nc
        carrier = nc.sync.nop()
        wait_clock.add_sem_waits(
            carrier.ins, ScopedClock({None: tick_clock.global_clock})
        )
        waits = list(carrier.ins.sync_info.on_wait)
        if len(waits) > 1:
            del carrier.ins.sync_info.on_wait[1:]
            for w in waits[1:]:
                extra = nc.sync.nop()
                extra.ins.sync_info = mybir.SyncInfo(on_wait=[w], on_update=[])
        nc.sync.drain()

        join = nc.alloc_semaphore("tail_join")
        nc.tensor.nop().then_inc(join, 1)
        nc.vector.nop().then_inc(join, 1)
        nc.scalar.nop().then_inc(join, 1)
        nc.sync.nop().then_inc(join, 1)
        nc.gpsimd.wait_ge(join, 4)

        popped = nc._tile_sem_poison_stack.pop()
        assert popped is self._sem_poison
        nc.clear_and_free_semaphores(list(self.sems.allocated().values()))
        nc.gpsimd.sem_clear(range(join.num, join.num + 1))

    tile.TileContext._drain_and_barrier = _drain_and_barrier
    tile.TileContext._drain_patch_applied = True


def _split_multi_waits(nc, mybir, max_waits=1):
    """This walrus build encodes at most one sync-wait per instruction
    (CTRL_NO and S3_LW structs both overflow at 2). Hoist extra waits
    onto same-engine NOPs placed immediately before the instruction —
    the engine's sequencer executes them in order, so the semantics are
    identical."""
    for fn in nc.m.functions:
        for blk in fn.blocks:
            il = blk.instructions
            new_il = []
            for inst in il:
                si = inst.sync_info
                if si is not None and len(si.on_wait) > max_waits:
                    for w in si.on_wait[max_waits:]:
                        nop = mybir.InstNoOp(name=f"I-{nc.next_id()}", ins=[], outs=[])
                        nop.engine = inst.engine
                        nop.sync_info = mybir.SyncInfo(on_wait=[w], on_update=[])
                        new_il.append(nop)
                    del si.on_wait[max_waits:]
                new_il.append(inst)
            il[:] = new_il


def _patch_ldw_opt():
    """Enable walrus's consecutive-LDWEIGHTS dedup (concourse pins it off).
    fc1/fc2 are ordered so same-stationary matmuls are adjacent, so the
    dedup removes most weight-load overhead. Routed through a shim that
    rewrites the flag, keeping the rest of the compile args intact."""
    import os
    import stat
    import tempfile
    import concourse.bass_utils as bu

    if getattr(bu, "_ldw_shim_applied", False):
        return
    real = bu.get_walrus_driver()
    shim_dir = tempfile.mkdtemp(prefix="walrus_shim_")
    shim = os.path.join(shim_dir, "walrus_driver")
    with open(shim, "w") as f:
        f.write(
            "#!/bin/bash\nargs=()\nfor a in \"$@\"; do\n"
            "  if [ \"$a\" = --enable-ldw-opt=false ]; then a=--enable-ldw-opt=true; fi\n"
            "  args+=(\"$a\")\ndone\n"
            f"exec {real} \"${{args[@]}}\"\n"
        )
    os.chmod(shim, os.stat(shim).st_mode | stat.S_IEXEC)
    bu.get_walrus_driver = lambda: shim
    bu._ldw_shim_applied = True


def _build_program():
    import concourse.bass as bass
    import concourse.mybir as mybir
    import concourse.tile as tile

    _patch_tile_drain()
    _patch_ldw_opt()

    f32 = mybir.dt.float32
    f32r = mybir.dt.float32r
    AF = mybir.ActivationFunctionType
    OP = mybir.AluOpType
    AX = mybir.AxisListType

    nc = bass.Bass()

    xT_d = nc.declare_dram_parameter("xT", [D, T], f32r, isOutput=False)
    w1t_d = nc.declare_dram_parameter("w1t", [D, H], f32r, isOutput=False)
    w2t_d = nc.declare_dram_parameter("w2t", [P, NH * D], f32r, isOutput=False)
    smalls_d = nc.declare_dram_parameter("smalls", [P, 48], f32r, isOutput=False)
    partial_d = nc.declare_dram_parameter("partial", [T, D], f32, isOutput=True)
    logits_d = nc.declare_dram_parameter("logits", [T, E], f32, isOutput=True)

    with tile.TileContext(nc) as tc:
        with (
            tc.tile_pool(name="consts", bufs=1) as consts,
            tc.tile_pool(name="ht", bufs=1) as htp,
            tc.tile_pool(name="small", bufs=4) as small,
            tc.tile_pool(name="outp", bufs=3) as outp,
            tc.tile_pool(name="ps_fc1", bufs=4, space="PSUM") as ps_fc1,
        ):
            from contextlib import ExitStack as _ES
            # ---- persistent input tiles -------------------------------
            # DMA issue order matters: the HWDGE queue drains in order, so
            # small router inputs go first, then xt/w1t interleaved so the
            # router and fc1 can start ~3us in; w2t (fc2-only) goes last.
            smalls = consts.tile([P, 48], f32r, name="smalls", tag="smalls")
            nc.sync.dma_start(smalls[:], smalls_d[:, :])
            wrt = [smalls[:, d * E:(d + 1) * E] for d in range(ND)]
            bb = smalls[:, 32:40].bitcast(f32)
            id8 = smalls[0:E, 40:48].bitcast(f32)
            xt = []
            w1t = []
            for d in range(ND):
                t_ = consts.tile([P, T], f32r, name=f"xt{d}", tag=f"xt{d}")
                nc.sync.dma_start(t_[:], xT_d[d * P:(d + 1) * P, :])
                xt.append(t_)
                t2_ = consts.tile([P, H], f32r, name=f"w1t{d}", tag=f"w1t{d}")
                nc.sync.dma_start(t2_[:], w1t_d[d * P:(d + 1) * P, :])
                w1t.append(t2_)

            w2t_all = consts.tile([P, NH, D], f32r, name="w2ta", tag="w2ta")
            nc.sync.dma_start(w2t_all[:], w2t_d[:, :])
            w2t = [w2t_all[:, h, :] for h in range(NH)]

            # ---- router + dispatch coefficient per token subtile ------
            # dispatch(t) for this core's expert (= permuted column 0):
            #   m1 = max_e logit, m2 = 2nd max
            #   d1 = sigmoid(m1-m2) = 0.5*tanh((m1-m2)/2)+0.5 ; d2 = 1-d1
            #   c  = (l0==m1)*d1 + (l0==m2)*d2
            # Router matmul with w_router.T stationary (M=8): out is
            # logitsT [8, 512] per token chunk; PE-transpose 128-token
            # slices back to [128, 8] so all per-token math stays on the
            # per-partition fast path. All 16 transposed results live in
            # one shared PSUM bank (8 columns each).
            # Router PSUM pools live only for the router phase; closing
            # them frees their banks for a third fc2 bank (8-bank budget:
            # fc1 4 + ps_r 2 + ps_tr 1 = 7, then fc1 4 + fc2 3 = 7).
            _rstack = _ES()
            ps_r = _rstack.enter_context(
                tc.tile_pool(name="ps_r", bufs=2, space="PSUM")
            )
            ps_tr = _rstack.enter_context(
                tc.tile_pool(name="ps_tr", bufs=2, space="PSUM")
            )
            lgT_tiles = []
            for t in range(NT):
                prT = ps_r.tile([E, TCH], f32, name="rpsumT", tag="rpsumT")
                for d in range(ND):
                    nc.tensor.matmul(
                        prT[:],
                        wrt[d],
                        xt[d][:, t * TCH:(t + 1) * TCH],
                        start=(d == 0),
                        stop=(d == ND - 1),
                    )
                lgT = small.tile([E, TCH], f32, name="lgT", tag="lgT")
                nc.scalar.copy(lgT[:], prT[:])
                lgT_tiles.append(lgT)

            call_t = consts.tile([P, NTS], f32, name="call", tag="call")
            for s in range(NTS):
                t, q = divmod(s, TCH // P)
                ptr_t = ps_tr.tile([P, E], f32, name="trpsum", tag="trpsum")
                ptr = ptr_t[:]
                nc.tensor.transpose(
                    ptr, lgT_tiles[t][:, q * P:(q + 1) * P], id8
                )
                lg = small.tile([P, E], f32, name="lg", tag="lg")
                nc.vector.tensor_add(lg[:], ptr, bb)
                nc.sync.dma_start(logits_d[s * P:(s + 1) * P, :], lg[:])
                rsc = small.tile([P, 8], f32, name="rsc", tag="rsc")
                m1, m2, dl, sg = rsc[:, 0:1], rsc[:, 1:2], rsc[:, 2:3], rsc[:, 3:4]
                d1, d2, e1, e2 = rsc[:, 4:5], rsc[:, 5:6], rsc[:, 6:7], rsc[:, 7:8]
                nc.vector.tensor_reduce(m1, lg[:], axis=AX.X, op=OP.max)
                msk = small.tile([P, E], f32, name="msk", tag="msk")
                nc.vector.tensor_scalar(
                    msk[:], lg[:], m1, -1e30, op0=OP.is_equal, op1=OP.mult
                )
                nc.vector.tensor_add(msk[:], msk[:], lg[:])
                nc.vector.tensor_reduce(m2, msk[:], axis=AX.X, op=OP.max)
                nc.vector.tensor_sub(dl, m1, m2)
                nc.scalar.activation(sg, dl, AF.Tanh, scale=0.5)
                nc.vector.tensor_scalar(d1, sg, 0.5, 0.5, op0=OP.mult, op1=OP.add)
                nc.vector.tensor_scalar(d2, sg, -0.5, 0.5, op0=OP.mult, op1=OP.add)
                nc.vector.tensor_scalar(e1, lg[:, 0:1], m1, None, op0=OP.is_equal)
                nc.vector.tensor_scalar(e2, lg[:, 0:1], m2, None, op0=OP.is_equal)
                nc.vector.tensor_mul(e1, e1, d1)
                nc.vector.tensor_mul(e2, e2, d2)
                nc.vector.tensor_add(call_t[:, s:s + 1], e1, e2)

            _rstack.close()
            _fstack = _ES()
            ps_fc2 = _fstack.enter_context(
                tc.tile_pool(name="ps_fc2", bufs=3, space="PSUM")
            )

            # ---- fc1: hT[h,t] = gelu(w1[e] @ x.T) ---------------------
            # lhsT = w1t chunk [128d, 128h]; rhs = xT chunk [128d, 512t]
            # h-outer / d / t-inner: one weight load per (h,d), streamed
            # over all 4 token chunks.
            ht = []
            for h in range(NH):
                hten = htp.tile([P, T], f32r, name=f"ht{h}", tag=f"ht{h}")
                ht.append(hten)
                psums = []
                for t in range(NT):
                    pt = ps_fc1.tile([P, TCH], f32, name="fc1psum", tag="fc1psum")
                    psums.append(pt)
                for d in range(ND):
                    for t in range(NT):
                        nc.tensor.matmul(
                            psums[t][:],
                            w1t[d][:, h * P:(h + 1) * P],
                            xt[d][:, t * TCH:(t + 1) * TCH],
                            start=(d == 0),
                            stop=(d == ND - 1),
                        )
                for t in range(NT):
                    nc.scalar.activation(
                        hten[:, t * TCH:(t + 1) * TCH], psums[t][:], AF.Gelu
                    )

            # ---- fc2 + dispatch-weighted combine ----------------------
            # lhsT = hT subtile [128h, 128t]; rhs = w2t chunk [128h, 512d]
            for t in range(NT):
                for q in range(TCH // P):
                    s = t * (TCH // P) + q
                    py = ps_fc2.tile([P, D], f32, name="fc2psum", tag="fc2psum")
                    for h in range(NH):
                        nc.tensor.matmul(
                            py[:],
                            ht[h][:, s * P:(s + 1) * P],
                            w2t[h],
                            start=(h == 0),
                            stop=(h == NH - 1),
                        )
                    ob = outp.tile([P, D], f32, name="ob", tag="ob")
                    nc.vector.tensor_scalar_mul(
                        ob[:], py[:], call_t[:, s:s + 1]
                    )
                    nc.sync.dma_start(partial_d[s * P:(s + 1) * P, :], ob[:])
            _fstack.close()

    _split_multi_waits(nc, mybir)
    return nc


def _prep_in_maps(x, w_router, b_router, w1, w2):
    x = np.ascontiguousarray(np.asarray(x, dtype=np.float32))
    w_router = np.ascontiguousarray(np.asarray(w_router, dtype=np.float32))
    b_router = np.ascontiguousarray(np.asarray(b_router, dtype=np.float32))
    w1 = np.ascontiguousarray(np.asarray(w1, dtype=np.float32))
    w2 = np.ascontiguousarray(np.asarray(w2, dtype=np.float32))

    xT = np.ascontiguousarray(x.reshape(T, D).T)
    in_maps = []
    for e in range(N_CORES):
        perm = list(range(E))
        perm[0], perm[e] = perm[e], perm[0]
        wr_p = w_router[perm]           # [E, D] with rows 0<->e swapped
        b_p = b_router[perm]
        # smalls[p, d*8+e] = wr_p.T[d*128+p, e]; cols 32:40 = bias bcast;
        # rows 0:8 of cols 40:48 = identity for the PE transpose.
        smalls = np.zeros((P, 48), dtype=np.float32)
        smalls[:, 0:32] = (
            wr_p.T.reshape(ND, P, E).transpose(1, 0, 2).reshape(P, ND * E)
        )
        smalls[:, 32:40] = np.broadcast_to(b_p[None, :], (P, E))
        smalls[0:E, 40:48] = np.eye(E, dtype=np.float32)
        # w2t packed: [p, h*512+j] = w2[e].T[h*128+p, j]
        w2tp = (
            np.ascontiguousarray(w2[e].T)
            .reshape(NH, P, D).transpose(1, 0, 2).reshape(P, NH * D)
        )
        in_maps.append({
            "xT": xT,
            "w1t": np.ascontiguousarray(w1[e].T),
            "w2t": np.ascontiguousarray(w2tp),
            "smalls": smalls,
        })
    return in_maps


def kernel(x, w_router, b_router, w1, w2):
    from concourse.bass_utils import run_bass_kernel_spmd

    if "nc" not in _CACHE:
        _CACHE["nc"] = _build_program()
    nc = _CACHE["nc"]

    in_maps = _prep_in_maps(x, w_router, b_router, w1, w2)
    res = run_bass_kernel_spmd(nc, in_maps, list(range(N_CORES)))

    out = np.zeros((T, D), dtype=np.float32)
    for e in range(N_CORES):
        out += res.results[e]["partial"]
    out = out.reshape(B, S, D)
    logits = res.results[0]["logits"]
    return out, logits


# revision 30
# speedup vs baseline: 1.0327x; 1.0327x over previous
"""Trainium2 Bass kernel for nn_MixtureOfExperts_89172111000183.

Strategy: expert-parallel across the 8 NeuronCores (E == n_cores == 8).
Core e holds only expert e's weights (pre-transposed on the host so the
contraction dim lands on SBUF partitions) and computes, for all T=2048
tokens:

    logits = x @ w_router.T + b          (full router, fp32 matmul)
    top-2 dispatch coefficient for THIS core's expert via a max/2nd-max
    trick (softmax + top-k renorm collapses to a sigmoid of m1-m2)
    partial_e = dispatch_e * (gelu(x @ w1[e].T) @ w2[e].T)

The host sums the 8 partials (the all-reduce step of expert-parallel
MoE) and takes router logits from core 0. The MLP matmuls run as
float32r (full-rate fp32 streaming on the PE); the router matmul runs
plain fp32 so top-2 selection matches the fp32 reference.

Per-core dispatch uses a column-swap trick: core e receives w_router.T
with columns 0<->e swapped, so "my expert's logit" is always column 0
of the on-device logits; core 0's swap is the identity, so its logits
output (the only one kept) is unpermuted.
"""

import numpy as np

# Problem shapes (hardcoded per contract).
B, S, D, H, E = 2, 1024, 512, 1024, 8
T = B * S            # 2048 tokens
P = 128              # SBUF partitions
TCH = 512            # token chunk (matmul free dim)
NT = T // TCH        # 4 token chunks
NTS = T // P         # 16 token subtiles
ND = D // P          # 4 contraction chunks for fc1/router
NH = H // P          # 8 h chunks
N_CORES = 8

_CACHE = {}


def _patch_tile_drain():
    """This walrus build's CTRL_NO struct encodes at most one sync-wait,
    but TileContext's kernel-tail drain carries one wait per live
    semaphore. Split the waits onto single-wait NOPs before the drain."""
    import concourse.tile as tile
    import concourse.mybir as mybir
    from concourse.vector_clock import ScopedClock

    if getattr(tile.TileContext, "_drain_patch_applied", False):
        return

    def _drain_and_barrier(self, tick_clock, wait_clock):
        # Slim tail: the stock version runs TWO all-engine EVSEM
        # butterflies around the sem clears (~9-17us). Replace with a
        # single join semaphore: SP waits for every proc's final tick
        # (carrier nops cover all DMA-completion sems) and drains its
        # queues, each compute engine signals stream-end, and gpsimd
        # clears the semaphore range once all five engines have signaled
        # so the next execution of the NEFF starts from zeroed sems.
        nc = self.nc
        carrier = nc.sync.nop()
        wait_clock.add_sem_waits(
            carrier.ins, ScopedClock({None: tick_clock.global_clock})
        )
        waits = list(carrier.ins.sync_info.on_wait)
        if len(waits) > 1:
            del carrier.ins.sync_info.on_wait[1:]
            for w in waits[1:]:
                extra = nc.sync.nop()
                extra.ins.sync_info = mybir.SyncInfo(on_wait=[w], on_update=[])
        nc.sync.drain()

        join = nc.alloc_semaphore("tail_join")
        nc.tensor.nop().then_inc(join, 1)
        nc.vector.nop().then_inc(join, 1)
        nc.scalar.nop().then_inc(join, 1)
        nc.sync.nop().then_inc(join, 1)
        nc.gpsimd.wait_ge(join, 4)

        popped = nc._tile_sem_poison_stack.pop()
        assert popped is self._sem_poison
        nc.clear_and_free_semaphores(list(self.sems.allocated().values()))
        nc.gpsimd.sem_clear(range(join.num, join.num + 1))

    tile.TileContext._drain_and_barrier = _drain_and_barrier
    tile.TileContext._drain_patch_applied = True


def _split_multi_waits(nc, mybir, max_waits=1):
    """This walrus build encodes at most one sync-wait per instruction
    (CTRL_NO and S3_LW structs both overflow at 2). Hoist extra waits
    onto same-engine NOPs placed immediately before the instruction —
    the engine's sequencer executes them in order, so the semantics are
    identical."""
    for fn in nc.m.functions:
        for blk in fn.blocks:
            il = blk.instructions
            new_il = []
            for inst in il:
                si = inst.sync_info
                if si is not None and len(si.on_wait) > max_waits:
                    for w in si.on_wait[max_waits:]:
                        nop = mybir.InstNoOp(name=f"I-{nc.next_id()}", ins=[], outs=[])
                        nop.engine = inst.engine
                        nop.sync_info = mybir.SyncInfo(on_wait=[w], on_update=[])
                        new_il.append(nop)
                    del si.on_wait[max_waits:]
                new_il.append(inst)
            il[:] = new_il


def _patch_ldw_opt():
    """Enable walrus's consecutive-LDWEIGHTS dedup (concourse pins it off).
    fc1/fc2 are ordered so same-stationary matmuls are adjacent, so the
    dedup removes most weight-load overhead. Routed through a shim that
    rewrites the flag, keeping the rest of the compile args intact."""
    import os
    import stat
    import tempfile
    import concourse.bass_utils as bu

    if getattr(bu, "_ldw_shim_applied", False):
        return
    real = bu.get_walrus_driver()
    shim_dir = tempfile.mkdtemp(prefix="walrus_shim_")
    shim = os.path.join(shim_dir, "walrus_driver")
    with open(shim, "w") as f:
        f.write(
            "#!/bin/bash\nargs=()\nfor a in \"$@\"; do\n"
            "  if [ \"$a\" = --enable-ldw-opt=false ]; then a=--enable-ldw-opt=true; fi\n"
            "  args+=(\"$a\")\ndone\n"
            f"exec {real} \"${{args[@]}}\"\n"
        )
    os.chmod(shim, os.stat(shim).st_mode | stat.S_IEXEC)
    bu.get_walrus_driver = lambda: shim
    bu._ldw_shim_applied = True


def _build_program():
    import concourse.bass as bass
    import concourse.mybir as mybir
    import concourse.tile as tile

    _patch_tile_drain()
    _patch_ldw_opt()

    f32 = mybir.dt.float32
    f32r = mybir.dt.float32r
    AF = mybir.ActivationFunctionType
    OP = mybir.AluOpType
    AX = mybir.AxisListType

    nc = bass.Bass()

    xT_d = nc.declare_dram_parameter("xT", [D, T], f32r, isOutput=False)
    w1t_d = nc.declare_dram_parameter("w1t", [D, H], f32r, isOutput=False)
    w2t_d = nc.declare_dram_parameter("w2t", [P, NH * D], f32r, isOutput=False)
    smalls_d = nc.declare_dram_parameter("smalls", [P, 48], f32r, isOutput=False)
    partial_d = nc.declare_dram_parameter("partial", [T, D], f32, isOutput=True)
    logits_d = nc.declare_dram_parameter("logits", [T, E], f32, isOutput=True)

    with tile.TileContext(nc) as tc:
        with (
            tc.tile_pool(name="consts", bufs=1) as consts,
            tc.tile_pool(name="ht", bufs=1) as htp,
            tc.tile_pool(name="small", bufs=4) as small,
            tc.tile_pool(name="outp", bufs=3) as outp,
            tc.tile_pool(name="ps_fc1", bufs=4, space="PSUM") as ps_fc1,
        ):
            from contextlib import ExitStack as _ES
            # ---- persistent input tiles -------------------------------
            # DMA issue order matters: the HWDGE queue drains in order, so
            # small router inputs go first, then xt/w1t interleaved so the
            # router and fc1 can start ~3us in; w2t (fc2-only) goes last.
            smalls = consts.tile([P, 48], f32r, name="smalls", tag="smalls")
            nc.sync.dma_start(smalls[:], smalls_d[:, :])
            wrt = [smalls[:, d * E:(d + 1) * E] for d in range(ND)]
            bb = smalls[:, 32:40].bitcast(f32)
            id8 = smalls[0:E, 40:48].bitcast(f32)
            xt = []
            w1t = []
            for d in range(ND):
                t_ = consts.tile([P, T], f32r, name=f"xt{d}", tag=f"xt{d}")
                nc.sync.dma_start(t_[:], xT_d[d * P:(d + 1) * P, :])
                xt.append(t_)
                t2_ = consts.tile([P, H], f32r, name=f"w1t{d}", tag=f"w1t{d}")
                nc.sync.dma_start(t2_[:], w1t_d[d * P:(d + 1) * P, :])
                w1t.append(t2_)

            w2t_all = consts.tile([P, NH, D], f32r, name="w2ta", tag="w2ta")
            nc.sync.dma_start(w2t_all[:], w2t_d[:, :])
            w2t = [w2t_all[:, h, :] for h in range(NH)]

            # ---- router + dispatch coefficient per token subtile ------
            # dispatch(t) for this core's expert (= permuted column 0):
            #   m1 = max_e logit, m2 = 2nd max
            #   d1 = sigmoid(m1-m2) = 0.5*tanh((m1-m2)/2)+0.5 ; d2 = 1-d1
            #   c  = (l0==m1)*d1 + (l0==m2)*d2
            # Router matmul with w_router.T stationary (M=8): out is
            # logitsT [8, 512] per token chunk; PE-transpose 128-token
            # slices back to [128, 8] so all per-token math stays on the
            # per-partition fast path. All 16 transposed results live in
            # one shared PSUM bank (8 columns each).
            # Router PSUM pools live only for the router phase; closing
            # them frees their banks for a third fc2 bank (8-bank budget:
            # fc1 4 + ps_r 2 + ps_tr 1 = 7, then fc1 4 + fc2 3 = 7).
            _rstack = _ES()
            ps_r = _rstack.enter_context(
                tc.tile_pool(name="ps_r", bufs=2, space="PSUM")
            )
            ps_tr = _rstack.enter_context(
                tc.tile_pool(name="ps_tr", bufs=1, space="PSUM")
            )
            ptr_all = ps_tr.tile([P, NTS * E], f32, name="trpsum", tag="trpsum")
            lgT_tiles = []
            for t in range(NT):
                prT = ps_r.tile([E, TCH], f32, name="rpsumT", tag="rpsumT")
                for d in range(ND):
                    nc.tensor.matmul(
                        prT[:],
                        wrt[d],
                        xt[d][:, t * TCH:(t + 1) * TCH],
                        start=(d == 0),
                        stop=(d == ND - 1),
                    )
                lgT = small.tile([E, TCH], f32, name="lgT", tag="lgT")
                nc.scalar.copy(lgT[:], prT[:])
                lgT_tiles.append(lgT)

            call_t = consts.tile([P, NTS], f32, name="call", tag="call")
            for s in range(NTS):
                t, q = divmod(s, TCH // P)
                ptr = ptr_all[:, s * E:(s + 1) * E]
                nc.tensor.transpose(
                    ptr, lgT_tiles[t][:, q * P:(q + 1) * P], id8
                )
                lg = small.tile([P, E], f32, name="lg", tag="lg")
                nc.vector.tensor_add(lg[:], ptr, bb)
                nc.sync.dma_start(logits_d[s * P:(s + 1) * P, :], lg[:])
                rsc = small.tile([P, 8], f32, name="rsc", tag="rsc")
                m1, m2, dl, sg = rsc[:, 0:1], rsc[:, 1:2], rsc[:, 2:3], rsc[:, 3:4]
                d1, d2, e1, e2 = rsc[:, 4:5], rsc[:, 5:6], rsc[:, 6:7], rsc[:, 7:8]
                nc.vector.tensor_reduce(m1, lg[:], axis=AX.X, op=OP.max)
                msk = small.tile([P, E], f32, name="msk", tag="msk")
                nc.vector.tensor_scalar(
                    msk[:], lg[:], m1, -1e30, op0=OP.is_equal, op1=OP.mult
                )
                nc.vector.tensor_add(msk[:], msk[:], lg[:])
                nc.vector.tensor_reduce(m2, msk[:], axis=AX.X, op=OP.max)
                nc.vector.tensor_sub(dl, m1, m2)
                nc.scalar.activation(sg, dl, AF.Tanh, scale=0.5)
                nc.vector.tensor_scalar(d1, sg, 0.5, 0.5, op0=OP.mult, op1=OP.add)
                nc.vector.tensor_scalar(d2, sg, -0.5, 0.5, op0=OP.mult, op1=OP.add)
                nc.vector.tensor_scalar(e1, lg[:, 0:1], m1, None, op0=OP.is_equal)
                nc.vector.tensor_scalar(e2, lg[:, 0:1], m2, None, op0=OP.is_equal)
                nc.vector.tensor_mul(e1, e1, d1)
                nc.vector.tensor_mul(e2, e2, d2)
                nc.vector.tensor_add(call_t[:, s:s + 1], e1, e2)

            _rstack.close()
            _fstack = _ES()
            ps_fc2 = _fstack.enter_context(
                tc.tile_pool(name="ps_fc2", bufs=3, space="PSUM")
            )

            # ---- fc1: hT[h,t] = gelu(w1[e] @ x.T) ---------------------
            # lhsT = w1t chunk [128d, 128h]; rhs = xT chunk [128d, 512t]
            # h-outer / d / t-inner: one weight load per (h,d), streamed
            # over all 4 token chunks.
            ht = []
            for h in range(NH):
                hten = htp.tile([P, T], f32r, name=f"ht{h}", tag=f"ht{h}")
                ht.append(hten)
                psums = []
                for t in range(NT):
                    pt = ps_fc1.tile([P, TCH], f32, name="fc1psum", tag="fc1psum")
                    psums.append(pt)
                for d in range(ND):
                    for t in range(NT):
                        nc.tensor.matmul(
                            psums[t][:],
                            w1t[d][:, h * P:(h + 1) * P],
                            xt[d][:, t * TCH:(t + 1) * TCH],
                            start=(d == 0),
                            stop=(d == ND - 1),
                        )
                for t in range(NT):
                    nc.scalar.activation(
                        hten[:, t * TCH:(t + 1) * TCH], psums[t][:], AF.Gelu
                    )

            # ---- fc2 + dispatch-weighted combine ----------------------
            # lhsT = hT subtile [128h, 128t]; rhs = w2t chunk [128h, 512d]
            for t in range(NT):
                for q in range(TCH // P):
                    s = t * (TCH // P) + q
                    py = ps_fc2.tile([P, D], f32, name="fc2psum", tag="fc2psum")
                    for h in range(NH):
                        nc.tensor.matmul(
                            py[:],
                            ht[h][:, s * P:(s + 1) * P],
                            w2t[h],
                            start=(h == 0),
                            stop=(h == NH - 1),
                        )
                    ob = outp.tile([P, D], f32, name="ob", tag="ob")
                    nc.vector.tensor_scalar_mul(
                        ob[:], py[:], call_t[:, s:s + 1]
                    )
                    nc.sync.dma_start(partial_d[s * P:(s + 1) * P, :], ob[:])
            _fstack.close()

    _split_multi_waits(nc, mybir)
    return nc


def _prep_in_maps(x, w_router, b_router, w1, w2):
    x = np.ascontiguousarray(np.asarray(x, dtype=np.float32))
    w_router = np.ascontiguousarray(np.asarray(w_router, dtype=np.float32))
    b_router = np.ascontiguousarray(np.asarray(b_router, dtype=np.float32))
    w1 = np.ascontiguousarray(np.asarray(w1, dtype=np.float32))
    w2 = np.ascontiguousarray(np.asarray(w2, dtype=np.float32))

    xT = np.ascontiguousarray(x.reshape(T, D).T)
    in_maps = []
    for e in range(N_CORES):
        perm = list(range(E))
        perm[0], perm[e] = perm[e], perm[0]
        wr_p = w_router[perm]           # [E, D] with rows 0<->e swapped
        b_p = b_router[perm]
        # smalls[p, d*8+e] = wr_p.T[d*128+p, e]; cols 32:40 = bias bcast;
        # rows 0:8 of cols 40:48 = identity for the PE transpose.
        smalls = np.zeros((P, 48), dtype=np.float32)
        smalls[:, 0:32] = (
            wr_p.T.reshape(ND, P, E).transpose(1, 0, 2).reshape(P, ND * E)
        )
        smalls[:, 32:40] = np.broadcast_to(b_p[None, :], (P, E))
        smalls[0:E, 40:48] = np.eye(E, dtype=np.float32)
        # w2t packed: [p, h*512+j] = w2[e].T[h*128+p, j]
        w2tp = (
            np.ascontiguousarray(w2[e].T)
            .reshape(NH, P, D).transpose(1, 0, 2).reshape(P, NH * D)
        )
        in_maps.append({
            "xT": xT,
            "w1t": np.ascontiguousarray(w1[e].T),
            "w2t": np.ascontiguousarray(w2tp),
            "smalls": smalls,
        })
    return in_maps


def kernel(x, w_router, b_router, w1, w2):
    from concourse.bass_utils import run_bass_kernel_spmd

    if "nc" not in _CACHE:
        _CACHE["nc"] = _build_program()
    nc = _CACHE["nc"]

    in_maps = _prep_in_maps(x, w_router, b_router, w1, w2)
    res = run_bass_kernel_spmd(nc, in_maps, list(range(N_CORES)))

    out = np.zeros((T, D), dtype=np.float32)
    for e in range(N_CORES):
        out += res.results[e]["partial"]
    out = out.reshape(B, S, D)
    logits = res.results[0]["logits"]
    return out, logits


# revision 31
# speedup vs baseline: 1.0343x; 1.0015x over previous
"""Trainium2 Bass kernel for nn_MixtureOfExperts_89172111000183.

Strategy: expert-parallel across the 8 NeuronCores (E == n_cores == 8).
Core e holds only expert e's weights (pre-transposed on the host so the
contraction dim lands on SBUF partitions) and computes, for all T=2048
tokens:

    logits = x @ w_router.T + b          (full router, fp32 matmul)
    top-2 dispatch coefficient for THIS core's expert via a max/2nd-max
    trick (softmax + top-k renorm collapses to a sigmoid of m1-m2)
    partial_e = dispatch_e * (gelu(x @ w1[e].T) @ w2[e].T)

The host sums the 8 partials (the all-reduce step of expert-parallel
MoE) and takes router logits from core 0. The MLP matmuls run as
float32r (full-rate fp32 streaming on the PE); the router matmul runs
plain fp32 so top-2 selection matches the fp32 reference.

Per-core dispatch uses a column-swap trick: core e receives w_router.T
with columns 0<->e swapped, so "my expert's logit" is always column 0
of the on-device logits; core 0's swap is the identity, so its logits
output (the only one kept) is unpermuted.
"""

import numpy as np

# Problem shapes (hardcoded per contract).
B, S, D, H, E = 2, 1024, 512, 1024, 8
T = B * S            # 2048 tokens
P = 128              # SBUF partitions
TCH = 512            # token chunk (matmul free dim)
NT = T // TCH        # 4 token chunks
NTS = T // P         # 16 token subtiles
ND = D // P          # 4 contraction chunks for fc1/router
NH = H // P          # 8 h chunks
N_CORES = 8

_CACHE = {}


def _patch_tile_drain():
    """This walrus build's CTRL_NO struct encodes at most one sync-wait,
    but TileContext's kernel-tail drain carries one wait per live
    semaphore. Split the waits onto single-wait NOPs before the drain."""
    import concourse.tile as tile
    import concourse.mybir as mybir
    from concourse.vector_clock import ScopedClock

    if getattr(tile.TileContext, "_drain_patch_applied", False):
        return

    def _drain_and_barrier(self, tick_clock, wait_clock):
        # Slim tail: the stock version runs TWO all-engine EVSEM
        # butterflies around the sem clears (~9-17us). Replace with a
        # single join semaphore: SP waits for every proc's final tick
        # (carrier nops cover all DMA-completion sems) and drains its
        # queues, each compute engine signals stream-end, and gpsimd
        # clears the semaphore range once all five engines have signaled
        # so the next execution of the NEFF starts from zeroed sems.
        nc = self.nc
        carrier = nc.sync.nop()
        wait_clock.add_sem_waits(
            carrier.ins, ScopedClock({None: tick_clock.global_clock})
        )
        waits = list(carrier.ins.sync_info.on_wait)
        if len(waits) > 1:
            del carrier.ins.sync_info.on_wait[1:]
            for w in waits[1:]:
                extra = nc.sync.nop()
                extra.ins.sync_info = mybir.SyncInfo(on_wait=[w], on_update=[])
        nc.sync.drain()

        join = nc.alloc_semaphore("tail_join")
        nc.tensor.nop().then_inc(join, 1)
        nc.vector.nop().then_inc(join, 1)
        nc.scalar.nop().then_inc(join, 1)
        nc.sync.nop().then_inc(join, 1)
        nc.gpsimd.wait_ge(join, 4)

        popped = nc._tile_sem_poison_stack.pop()
        assert popped is self._sem_poison
        nc.clear_and_free_semaphores(list(self.sems.allocated().values()))
        nc.gpsimd.sem_clear(range(join.num, join.num + 1))

    tile.TileContext._drain_and_barrier = _drain_and_barrier
    tile.TileContext._drain_patch_applied = True


def _split_multi_waits(nc, mybir, max_waits=1):
    """This walrus build encodes at most one sync-wait per instruction
    (CTRL_NO and S3_LW structs both overflow at 2). Hoist extra waits
    onto same-engine NOPs placed immediately before the instruction —
    the engine's sequencer executes them in order, so the semantics are
    identical."""
    for fn in nc.m.functions:
        for blk in fn.blocks:
            il = blk.instructions
            new_il = []
            for inst in il:
                si = inst.sync_info
                if si is not None and len(si.on_wait) > max_waits:
                    for w in si.on_wait[max_waits:]:
                        nop = mybir.InstNoOp(name=f"I-{nc.next_id()}", ins=[], outs=[])
                        nop.engine = inst.engine
                        nop.sync_info = mybir.SyncInfo(on_wait=[w], on_update=[])
                        new_il.append(nop)
                    del si.on_wait[max_waits:]
                new_il.append(inst)
            il[:] = new_il


def _patch_ldw_opt():
    """Enable walrus's consecutive-LDWEIGHTS dedup (concourse pins it off).
    fc1/fc2 are ordered so same-stationary matmuls are adjacent, so the
    dedup removes most weight-load overhead. Routed through a shim that
    rewrites the flag, keeping the rest of the compile args intact."""
    import os
    import stat
    import tempfile
    import concourse.bass_utils as bu

    if getattr(bu, "_ldw_shim_applied", False):
        return
    real = bu.get_walrus_driver()
    shim_dir = tempfile.mkdtemp(prefix="walrus_shim_")
    shim = os.path.join(shim_dir, "walrus_driver")
    with open(shim, "w") as f:
        f.write(
            "#!/bin/bash\nargs=()\nfor a in \"$@\"; do\n"
            "  if [ \"$a\" = --enable-ldw-opt=false ]; then a=--enable-ldw-opt=true; fi\n"
            "  args+=(\"$a\")\ndone\n"
            f"exec {real} \"${{args[@]}}\"\n"
        )
    os.chmod(shim, os.stat(shim).st_mode | stat.S_IEXEC)
    bu.get_walrus_driver = lambda: shim
    bu._ldw_shim_applied = True


def _build_program():
    import concourse.bass as bass
    import concourse.mybir as mybir
    import concourse.tile as tile

    _patch_tile_drain()
    _patch_ldw_opt()

    f32 = mybir.dt.float32
    f32r = mybir.dt.float32r
    AF = mybir.ActivationFunctionType
    OP = mybir.AluOpType
    AX = mybir.AxisListType

    nc = bass.Bass()

    xT_d = nc.declare_dram_parameter("xT", [D, T], f32r, isOutput=False)
    w1t_d = nc.declare_dram_parameter("w1t", [D, H], f32r, isOutput=False)
    w2t_d = nc.declare_dram_parameter("w2t", [P, NH * D], f32r, isOutput=False)
    smalls_d = nc.declare_dram_parameter("smalls", [P, 48], f32r, isOutput=False)
    partial_d = nc.declare_dram_parameter("partial", [T, D], f32, isOutput=True)
    logits_d = nc.declare_dram_parameter("logits", [T, E], f32, isOutput=True)

    with tile.TileContext(nc) as tc:
        with (
            tc.tile_pool(name="consts", bufs=1) as consts,
            tc.tile_pool(name="ht", bufs=1) as htp,
            tc.tile_pool(name="small", bufs=4) as small,
            tc.tile_pool(name="outp", bufs=3) as outp,
            tc.tile_pool(name="ps_fc1", bufs=5, space="PSUM") as ps_fc1,
        ):
            from contextlib import ExitStack as _ES
            # ---- persistent input tiles -------------------------------
            # DMA issue order matters: the HWDGE queue drains in order, so
            # small router inputs go first, then xt/w1t interleaved so the
            # router and fc1 can start ~3us in; w2t (fc2-only) goes last.
            smalls = consts.tile([P, 48], f32r, name="smalls", tag="smalls")
            nc.sync.dma_start(smalls[:], smalls_d[:, :])
            wrt = [smalls[:, d * E:(d + 1) * E] for d in range(ND)]
            bb = smalls[:, 32:40].bitcast(f32)
            id8 = smalls[0:E, 40:48].bitcast(f32)
            xt = []
            w1t = []
            for d in range(ND):
                t_ = consts.tile([P, T], f32r, name=f"xt{d}", tag=f"xt{d}")
                nc.sync.dma_start(t_[:], xT_d[d * P:(d + 1) * P, :])
                xt.append(t_)
                t2_ = consts.tile([P, H], f32r, name=f"w1t{d}", tag=f"w1t{d}")
                nc.sync.dma_start(t2_[:], w1t_d[d * P:(d + 1) * P, :])
                w1t.append(t2_)

            w2t_all = consts.tile([P, NH, D], f32r, name="w2ta", tag="w2ta")
            nc.sync.dma_start(w2t_all[:], w2t_d[:, :])
            w2t = [w2t_all[:, h, :] for h in range(NH)]

            # ---- router + dispatch coefficient per token subtile ------
            # dispatch(t) for this core's expert (= permuted column 0):
            #   m1 = max_e logit, m2 = 2nd max
            #   d1 = sigmoid(m1-m2) = 0.5*tanh((m1-m2)/2)+0.5 ; d2 = 1-d1
            #   c  = (l0==m1)*d1 + (l0==m2)*d2
            # Router matmul with w_router.T stationary (M=8): out is
            # logitsT [8, 512] per token chunk; PE-transpose 128-token
            # slices back to [128, 8] so all per-token math stays on the
            # per-partition fast path. All 16 transposed results live in
            # one shared PSUM bank (8 columns each).
            # Router PSUM pools live only for the router phase; closing
            # them frees their banks for a third fc2 bank (8-bank budget:
            # fc1 4 + ps_r 2 + ps_tr 1 = 7, then fc1 4 + fc2 3 = 7).
            _rstack = _ES()
            ps_r = _rstack.enter_context(
                tc.tile_pool(name="ps_r", bufs=2, space="PSUM")
            )
            ps_tr = _rstack.enter_context(
                tc.tile_pool(name="ps_tr", bufs=1, space="PSUM")
            )
            ptr_all = ps_tr.tile([P, NTS * E], f32, name="trpsum", tag="trpsum")
            lgT_tiles = []
            for t in range(NT):
                prT = ps_r.tile([E, TCH], f32, name="rpsumT", tag="rpsumT")
                for d in range(ND):
                    nc.tensor.matmul(
                        prT[:],
                        wrt[d],
                        xt[d][:, t * TCH:(t + 1) * TCH],
                        start=(d == 0),
                        stop=(d == ND - 1),
                    )
                lgT = small.tile([E, TCH], f32, name="lgT", tag="lgT")
                nc.scalar.copy(lgT[:], prT[:])
                lgT_tiles.append(lgT)

            call_t = consts.tile([P, NTS], f32, name="call", tag="call")
            for s in range(NTS):
                t, q = divmod(s, TCH // P)
                ptr = ptr_all[:, s * E:(s + 1) * E]
                nc.tensor.transpose(
                    ptr, lgT_tiles[t][:, q * P:(q + 1) * P], id8
                )
                lg = small.tile([P, E], f32, name="lg", tag="lg")
                nc.vector.tensor_add(lg[:], ptr, bb)
                nc.sync.dma_start(logits_d[s * P:(s + 1) * P, :], lg[:])
                rsc = small.tile([P, 8], f32, name="rsc", tag="rsc")
                m1, m2, dl, sg = rsc[:, 0:1], rsc[:, 1:2], rsc[:, 2:3], rsc[:, 3:4]
                d1, d2, e1, e2 = rsc[:, 4:5], rsc[:, 5:6], rsc[:, 6:7], rsc[:, 7:8]
                nc.vector.tensor_reduce(m1, lg[:], axis=AX.X, op=OP.max)
                msk = small.tile([P, E], f32, name="msk", tag="msk")
                nc.vector.tensor_scalar(
                    msk[:], lg[:], m1, -1e30, op0=OP.is_equal, op1=OP.mult
                )
                nc.vector.tensor_add(msk[:], msk[:], lg[:])
                nc.vector.tensor_reduce(m2, msk[:], axis=AX.X, op=OP.max)
                nc.vector.tensor_sub(dl, m1, m2)
                nc.scalar.activation(sg, dl, AF.Tanh, scale=0.5)
                nc.vector.tensor_scalar(d1, sg, 0.5, 0.5, op0=OP.mult, op1=OP.add)
                nc.vector.tensor_scalar(d2, sg, -0.5, 0.5, op0=OP.mult, op1=OP.add)
                nc.vector.tensor_scalar(e1, lg[:, 0:1], m1, None, op0=OP.is_equal)
                nc.vector.tensor_scalar(e2, lg[:, 0:1], m2, None, op0=OP.is_equal)
                nc.vector.tensor_mul(e1, e1, d1)
                nc.vector.tensor_mul(e2, e2, d2)
                nc.vector.tensor_add(call_t[:, s:s + 1], e1, e2)

            _rstack.close()
            _fstack = _ES()
            ps_fc2 = _fstack.enter_context(
                tc.tile_pool(name="ps_fc2", bufs=3, space="PSUM")
            )

            # ---- fc1: hT[h,t] = gelu(w1[e] @ x.T) ---------------------
            # lhsT = w1t chunk [128d, 128h]; rhs = xT chunk [128d, 512t]
            # h-outer / d / t-inner: one weight load per (h,d), streamed
            # over all 4 token chunks.
            ht = []
            for h in range(NH):
                hten = htp.tile([P, T], f32r, name=f"ht{h}", tag=f"ht{h}")
                ht.append(hten)
                psums = []
                for t in range(NT):
                    pt = ps_fc1.tile([P, TCH], f32, name="fc1psum", tag="fc1psum")
                    psums.append(pt)
                for d in range(ND):
                    for t in range(NT):
                        nc.tensor.matmul(
                            psums[t][:],
                            w1t[d][:, h * P:(h + 1) * P],
                            xt[d][:, t * TCH:(t + 1) * TCH],
                            start=(d == 0),
                            stop=(d == ND - 1),
                        )
                for t in range(NT):
                    nc.scalar.activation(
                        hten[:, t * TCH:(t + 1) * TCH], psums[t][:], AF.Gelu
                    )

            # ---- fc2 + dispatch-weighted combine ----------------------
            # lhsT = hT subtile [128h, 128t]; rhs = w2t chunk [128h, 512d]
            for t in range(NT):
                for q in range(TCH // P):
                    s = t * (TCH // P) + q
                    py = ps_fc2.tile([P, D], f32, name="fc2psum", tag="fc2psum")
                    for h in range(NH):
                        nc.tensor.matmul(
                            py[:],
                            ht[h][:, s * P:(s + 1) * P],
                            w2t[h],
                            start=(h == 0),
                            stop=(h == NH - 1),
                        )
                    ob = outp.tile([P, D], f32, name="ob", tag="ob")
                    nc.vector.tensor_scalar_mul(
                        ob[:], py[:], call_t[:, s:s + 1]
                    )
                    nc.sync.dma_start(partial_d[s * P:(s + 1) * P, :], ob[:])
            _fstack.close()

    _split_multi_waits(nc, mybir)
    return nc


def _prep_in_maps(x, w_router, b_router, w1, w2):
    x = np.ascontiguousarray(np.asarray(x, dtype=np.float32))
    w_router = np.ascontiguousarray(np.asarray(w_router, dtype=np.float32))
    b_router = np.ascontiguousarray(np.asarray(b_router, dtype=np.float32))
    w1 = np.ascontiguousarray(np.asarray(w1, dtype=np.float32))
    w2 = np.ascontiguousarray(np.asarray(w2, dtype=np.float32))

    xT = np.ascontiguousarray(x.reshape(T, D).T)
    in_maps = []
    for e in range(N_CORES):
        perm = list(range(E))
        perm[0], perm[e] = perm[e], perm[0]
        wr_p = w_router[perm]           # [E, D] with rows 0<->e swapped
        b_p = b_router[perm]
        # smalls[p, d*8+e] = wr_p.T[d*128+p, e]; cols 32:40 = bias bcast;
        # rows 0:8 of cols 40:48 = identity for the PE transpose.
        smalls = np.zeros((P, 48), dtype=np.float32)
        smalls[:, 0:32] = (
            wr_p.T.reshape(ND, P, E).transpose(1, 0, 2).reshape(P, ND * E)
        )
        smalls[:, 32:40] = np.broadcast_to(b_p[None, :], (P, E))
        smalls[0:E, 40:48] = np.eye(E, dtype=np.float32)
        # w2t packed: [p, h*512+j] = w2[e].T[h*128+p, j]
        w2tp = (
            np.ascontiguousarray(w2[e].T)
            .reshape(NH, P, D).transpose(1, 0, 2).reshape(P, NH * D)
        )
        in_maps.append({
            "xT": xT,
            "w1t": np.ascontiguousarray(w1[e].T),
            "w2t": np.ascontiguousarray(w2tp),
            "smalls": smalls,
        })
    return in_maps


def kernel(x, w_router, b_router, w1, w2):
    from concourse.bass_utils import run_bass_kernel_spmd

    if "nc" not in _CACHE:
        _CACHE["nc"] = _build_program()
    nc = _CACHE["nc"]

    in_maps = _prep_in_maps(x, w_router, b_router, w1, w2)
    res = run_bass_kernel_spmd(nc, in_maps, list(range(N_CORES)))

    out = np.zeros((T, D), dtype=np.float32)
    for e in range(N_CORES):
        out += res.results[e]["partial"]
    out = out.reshape(B, S, D)
    logits = res.results[0]["logits"]
    return out, logits
